# revision 7
# baseline (speedup 1.0000x reference)
"""Trainium2 Bass kernel for nn_ChannelShuffle (topk_masking).

Reference computation (per sample i of N=80, c=2048 channels, hw=256):
  scores = s_ca[i]                       # [c]
  topk_idx = top_k(scores, S=512)        # sorted desc, stable ties
  j = (i + 1 + partner[i]) % N
  blend[k] = 0.7*x[i, topk_idx[k]] + 0.3*x[j, rand_index[i, k]]
  aug = x[i] with channels topk_idx[k] <- blend[k]
  out[orig slot] = x[i] * scores ; out[aug slot] = aug * scores
  slots: g=way*16+t -> orig row way*32+t, aug row way*32+16+t (way=g//16)

Strategy: data-parallel over the batch dim, 10 samples per core (8 cores).
Host does index-only prep (argsort topk, partner mapping, gather/scatter
index streams, scale vectors); the device does all tensor math per sample:
  y   = x * s                  -> orig slot        (scalar engine)
  aug = x * A                  -> aug slot          (vector engine)
        where A = 0.7*s on topk channels else s
  xq  = dma_gather(x_part, rand rows) * (0.3*s_topk)   (rank space)
  indirect scatter-ADD of xq onto the aug slot's topk rows (CCE add),
  ordered after the dense write by Tile's DRAM dependency tracking
  (both mechanisms verified on HW).
"""

import numpy as np

# problem constants (hardcoded per harness contract)
N = 80          # batch
C = 2048        # channels
E = 256         # h*w = 16*16
S = 512         # shuffle_num
NCORES = 8
NLOC = N // NCORES          # samples per core
P = 128                     # partitions
CH = C // P                 # 16 free-dim chunks per sample; ch = p*CH + chunk
NRK = S // P                # 4 rank chunks; rank r = n*128 + p
SW = S // 16                # 32 idx stream cols for dma_gather

_CACHE = {}


def _build(n_loc=NLOC, reps=1, bufs=3):
    import concourse.bacc as bacc
    import concourse.tile as tile
    from concourse import bass, mybir

    nc = bacc.Bacc("TRN2", target_bir_lowering=False, debug=False,
                   num_devices=NCORES)

    x_own = nc.dram_tensor("x_own", [n_loc * C, E], mybir.dt.float32,
                           kind="ExternalInput")
    x_part = nc.dram_tensor("x_part", [n_loc * C, E], mybir.dt.float32,
                            kind="ExternalInput")
    # sscl cols: 0:CH = s (ch = p*CH+c); CH:2*CH = A (0.7*s on topk else s);
    # 2*CH:2*CH+NRK = 0.3*s_topk at rank slot (p, n)
    sscl = nc.dram_tensor("sscl", [n_loc, P, 2 * CH + NRK], mybir.dt.float32,
                          kind="ExternalInput")
    # gidx: int16 dma_gather stream (16-wrapped, core-replicated) of partner
    # rows in x_part
    gidx = nc.dram_tensor("gidx", [n_loc, P, SW], mybir.dt.int16,
                          kind="ExternalInput")
    # oidx: scatter dest rows (C + topk_idx) at rank slot (p, n)
    oidx = nc.dram_tensor("oidx", [n_loc, P, NRK], mybir.dt.int32,
                          kind="ExternalInput")
    outs = [
        nc.dram_tensor(f"out{i}", [2 * C, E], mybir.dt.float32,
                       kind="ExternalOutput")
        for i in range(n_loc)
    ]

    FREE = CH * E  # 4096 f32 per partition

    with tile.TileContext(nc) as tc:
        with (
            tc.tile_pool(name="xp", bufs=bufs) as xpool,
            tc.tile_pool(name="ap", bufs=bufs) as apool,
            tc.tile_pool(name="gp", bufs=n_loc) as gpool,
            tc.tile_pool(name="sp", bufs=n_loc) as spool,
        ):
            for _rep in range(reps):
                # Phase A: all partner gathers up front so the Pool queue's
                # scatters (which wait on dense stores) never block a gather
                xq_tiles, sscl_tiles, oidx_tiles = [], [], []
                for i in range(n_loc):
                    sscl_sb = spool.tile([P, 2 * CH + NRK], mybir.dt.float32,
                                         tag="sscl")
                    nc.sync.dma_start(sscl_sb[:], sscl[i])
                    gidx_sb = spool.tile([P, SW], mybir.dt.int16, tag="gidx")
                    nc.sync.dma_start(gidx_sb[:], gidx[i])
                    oidx_sb = spool.tile([P, NRK], mybir.dt.int32, tag="oidx")
                    nc.sync.dma_start(oidx_sb[:], oidx[i])
                    # partner rows, rank space: slot (p, n) = rank n*128+p
                    xq_sb = gpool.tile([P, NRK * E], mybir.dt.float32)
                    nc.gpsimd.dma_gather(
                        out_ap=xq_sb[:].rearrange("p (n e) -> p n e", e=E),
                        in_ap=x_part[:],
                        idxs_ap=gidx_sb[:],
                        num_idxs=S,
                        num_idxs_reg=S,
                        elem_size=E,
                    )
                    # xq *= 0.3*s_topk (per rank slot)
                    for n in range(NRK):
                        nc.vector.tensor_scalar_mul(
                            xq_sb[:, n * E:(n + 1) * E],
                            xq_sb[:, n * E:(n + 1) * E],
                            sscl_sb[:, 2 * CH + n:2 * CH + n + 1],
                        )
                    xq_tiles.append(xq_sb)
                    sscl_tiles.append(sscl_sb)
                    oidx_tiles.append(oidx_sb)

                # Phase B: dense pipeline + terminal scatters
                for i in range(n_loc):
                    sscl_sb = sscl_tiles[i]
                    x_sb = xpool.tile([P, FREE], mybir.dt.float32)
                    nc.sync.dma_start(
                        x_sb[:],
                        x_own[i * C:(i + 1) * C].rearrange(
                            "(p c) e -> p (c e)", p=P),
                    )
                    # aug = x*A (vector engine), then y = x*s in place (scalar)
                    a_sb = apool.tile([P, FREE], mybir.dt.float32)
                    for cI in range(CH):
                        sl = slice(cI * E, (cI + 1) * E)
                        nc.vector.tensor_scalar_mul(
                            a_sb[:, sl], x_sb[:, sl],
                            sscl_sb[:, CH + cI:CH + cI + 1],
                        )
                        nc.scalar.activation(
                            x_sb[:, sl], x_sb[:, sl],
                            mybir.ActivationFunctionType.Copy,
                            scale=sscl_sb[:, cI:cI + 1],
                        )
                    nc.sync.dma_start(
                        outs[i][0:C].rearrange("(p c) e -> p (c e)", p=P),
                        x_sb[:],
                    )
                    nc.sync.dma_start(
                        outs[i][C:2 * C].rearrange("(p c) e -> p (c e)", p=P),
                        a_sb[:],
                    )
                    # scatter-ADD blend remainder over the aug slot's topk rows
                    for n in range(NRK):
                        nc.gpsimd.indirect_dma_start(
                            out=outs[i][:],
                            out_offset=bass.IndirectOffsetOnAxis(
                                ap=oidx_tiles[i][:, n:n + 1], axis=0
                            ),
                            in_=xq_tiles[i][:, n * E:(n + 1) * E],
                            in_offset=None,
                            bounds_check=2 * C - 1,
                            oob_is_err=False,
                            compute_op=mybir.AluOpType.add,
                        )

    nc.compile()
    return nc


def _get_nc(n_loc=NLOC, reps=1, bufs=3):
    key = (n_loc, reps, bufs)
    if key not in _CACHE:
        _CACHE[key] = _build(n_loc, reps, bufs)
    return _CACHE[key]


def _wrap16(stream):
    """[S] stream -> [P, S//16] int16 tile (16-wrapped, replicated per core)."""
    t = stream.reshape(S // 16, 16).T.astype(np.int16)     # [16, S//16]
    return np.tile(t, (8, 1))                              # [128, S//16]


def _prep(x, s_ca, rand_index, partner):
    """Host-side index/scale prep. Returns per-core input maps."""
    scores = np.asarray(s_ca, np.float32).reshape(N, C)
    x = np.ascontiguousarray(np.asarray(x, np.float32).reshape(N, C, E))
    rand_index = np.asarray(rand_index).astype(np.int64).reshape(N, S)
    partner = np.asarray(partner).astype(np.int64).reshape(N)

    # top-k (stable desc sort == jax.lax.top_k tie semantics)
    order = np.argsort(-scores, axis=1, kind="stable")
    topk = order[:, :S]                                    # [N, S]
    j = (np.arange(N) + 1 + partner) % N                   # partner sample

    rows = np.arange(N)
    i_loc = rows % NLOC
    s_topk = np.take_along_axis(scores, topk, axis=1)      # [N, S]

    a_v = scores.copy()
    np.put_along_axis(a_v, topk, np.float32(0.7) * s_topk, axis=1)

    sscl = np.concatenate([
        scores.reshape(N, P, CH),
        a_v.reshape(N, P, CH),
        (np.float32(0.3) * s_topk).reshape(N, NRK, P).transpose(0, 2, 1),
    ], axis=2).astype(np.float32)                          # [N, P, 2*CH+NRK]

    # partner gather stream (rank order): rows in x_part flat tensor
    st_part = (i_loc[:, None] * C + rand_index).astype(np.int64)   # [N, S]
    gidx = np.empty((N, P, SW), np.int16)
    for g in range(N):
        gidx[g] = _wrap16(st_part[g])

    # scatter rows at rank slot (p, n): C + topk_idx[g, n*128+p]
    oidx = (C + topk).reshape(N, NRK, P).transpose(0, 2, 1).astype(np.int32)

    in_maps = []
    for k in range(NCORES):
        sl = slice(k * NLOC, (k + 1) * NLOC)
        in_maps.append({
            "x_own": x[sl].reshape(NLOC * C, E),
            "x_part": np.ascontiguousarray(x[j[sl]]).reshape(NLOC * C, E),
            "sscl": np.ascontiguousarray(sscl[sl]),
            "gidx": np.ascontiguousarray(gidx[sl]),
            "oidx": np.ascontiguousarray(oidx[sl]),
        })
    return in_maps


def _assemble(results):
    """Map per-core out{i} [2C, E] tensors into the full [2N, C, 16, 16]."""
    full = np.empty((2 * N, C, 16, 16), np.float32)
    for k in range(NCORES):
        for il in range(NLOC):
            oc = results[k][f"out{il}"].reshape(2, C, 16, 16)
            g = k * NLOC + il
            way, t = g // 16, g % 16
            full[way * 32 + t] = oc[0]
            full[way * 32 + 16 + t] = oc[1]
    return full


def kernel(x, s_ca, rand_index, partner, shuffle_num, _trace=False):
    from concourse import bass_utils

    assert int(shuffle_num) == S
    in_maps = _prep(x, s_ca, rand_index, partner)
    nc = _get_nc()
    res = bass_utils.run_bass_kernel_spmd(
        nc, in_maps, core_ids=list(range(NCORES)), trace=_trace
    )
    out = _assemble(res.results)
    if _trace:
        return out, res
    return out


# revision 10
# speedup vs baseline: 1.0476x; 1.0476x over previous
"""Trainium2 Bass kernel for nn_ChannelShuffle (topk_masking).

Reference computation (per sample i of N=80, c=2048 channels, hw=256):
  scores = s_ca[i]                       # [c]
  topk_idx = top_k(scores, S=512)        # sorted desc, stable ties
  j = (i + 1 + partner[i]) % N
  blend[k] = 0.7*x[i, topk_idx[k]] + 0.3*x[j, rand_index[i, k]]
  aug = x[i] with channels topk_idx[k] <- blend[k]
  out[orig slot] = x[i] * scores ; out[aug slot] = aug * scores
  slots: g=way*16+t -> orig row way*32+t, aug row way*32+16+t (way=g//16)

Strategy: data-parallel over the batch dim, 10 samples per core (8 cores).
Host does index-only prep (argsort topk, partner mapping, gather/scatter
index streams, scale vectors); the device does all tensor math per sample:
  y   = x * s                  -> orig slot        (scalar engine)
  aug = x * A                  -> aug slot          (vector engine)
        where A = 0.7*s on topk channels else s
  xq  = dma_gather(x_part, rand rows) * (0.3*s_topk)   (rank space)
  indirect scatter-ADD of xq onto the aug slot's topk rows (CCE add),
  ordered after the dense write by Tile's DRAM dependency tracking
  (both mechanisms verified on HW).
"""

import numpy as np

# problem constants (hardcoded per harness contract)
N = 80          # batch
C = 2048        # channels
E = 256         # h*w = 16*16
S = 512         # shuffle_num
NCORES = 8
NLOC = N // NCORES          # samples per core
P = 128                     # partitions
CH = C // P                 # 16 free-dim chunks per sample; ch = p*CH + chunk
NRK = S // P                # 4 rank chunks; rank r = n*128 + p
SW = S // 16                # 32 idx stream cols for dma_gather

_CACHE = {}


def _build(n_loc=NLOC, reps=1, bufs=3, pe_merge=True):
    import concourse.bacc as bacc
    import concourse.tile as tile
    from concourse import bass, mybir

    nc = bacc.Bacc("TRN2", target_bir_lowering=False, debug=False,
                   num_devices=NCORES)

    x_own = nc.dram_tensor("x_own", [n_loc * C, E], mybir.dt.float32,
                           kind="ExternalInput")
    x_part = nc.dram_tensor("x_part", [n_loc * C, E], mybir.dt.float32,
                            kind="ExternalInput")
    # sscl cols: 0:CH = s (ch = p*CH+c); CH:2*CH = A (0.7*s on topk else s);
    # 2*CH:2*CH+NRK = 0.3*s_topk at rank slot (p, n)
    sscl = nc.dram_tensor("sscl", [n_loc, P, 2 * CH + 2 * NRK],
                          mybir.dt.float32, kind="ExternalInput")
    # gidx: int16 dma_gather stream (16-wrapped, core-replicated) of partner
    # rows in x_part
    gidx = nc.dram_tensor("gidx", [n_loc, P, SW], mybir.dt.int16,
                          kind="ExternalInput")
    # oidx: scatter dest rows (C + topk_idx) at rank slot (p, n)
    oidx = nc.dram_tensor("oidx", [n_loc, P, NRK], mybir.dt.int32,
                          kind="ExternalInput")
    outs = [
        nc.dram_tensor(f"out{i}", [2 * C, E], mybir.dt.float32,
                       kind="ExternalOutput")
        for i in range(n_loc)
    ]

    FREE = CH * E  # 4096 f32 per partition

    big_bufs = min(bufs, 2) if pe_merge else bufs
    with tile.TileContext(nc) as tc:
        with (
            tc.tile_pool(name="xp", bufs=big_bufs) as xpool,
            tc.tile_pool(name="yp", bufs=big_bufs) as ypool,
            tc.tile_pool(name="ap", bufs=big_bufs) as apool,
            tc.tile_pool(name="gp", bufs=bufs) as gpool,
            tc.tile_pool(name="sp", bufs=bufs) as spool,
            tc.tile_pool(name="scp", bufs=2) as scpool,
            tc.tile_pool(name="pp", bufs=4, space="PSUM") as ppool,
            tc.tile_pool(name="cp", bufs=1) as cpool,
        ):
            if pe_merge:
                # constant: iota 0..C-1 along free dim (exact in f32)
                iota_f = cpool.tile([P, C], mybir.dt.float32, tag="iof")
                nc.gpsimd.iota(iota_f[:], [[1, C]], channel_multiplier=0,
                               allow_small_or_imprecise_dtypes=True)

            for i in [i for _ in range(reps) for i in range(n_loc)]:
                x_sb = xpool.tile([P, FREE], mybir.dt.float32)
                nc.sync.dma_start(
                    x_sb[:],
                    x_own[i * C:(i + 1) * C].rearrange("(p c) e -> p (c e)", p=P),
                )
                sscl_sb = spool.tile([P, 2 * CH + 2 * NRK], mybir.dt.float32)
                nc.sync.dma_start(sscl_sb[:], sscl[i])
                gidx_sb = spool.tile([P, SW], mybir.dt.int16, tag="gidx")
                nc.sync.dma_start(gidx_sb[:], gidx[i])
                if not pe_merge:
                    oidx_sb = spool.tile([P, NRK], mybir.dt.int32, tag="oidx")
                    nc.sync.dma_start(oidx_sb[:], oidx[i])

                # partner rows, rank space: slot (p, n) = rank n*128+p
                xq_sb = gpool.tile([P, NRK * E], mybir.dt.float32)
                nc.gpsimd.dma_gather(
                    out_ap=xq_sb[:].rearrange("p (n e) -> p n e", e=E),
                    in_ap=x_part[:],
                    idxs_ap=gidx_sb[:],
                    num_idxs=S,
                    num_idxs_reg=S,
                    elem_size=E,
                )
                # xq *= 0.3*s_topk (per rank slot)
                for n in range(NRK):
                    nc.vector.tensor_scalar_mul(
                        xq_sb[:, n * E:(n + 1) * E],
                        xq_sb[:, n * E:(n + 1) * E],
                        sscl_sb[:, 2 * CH + n:2 * CH + n + 1],
                    )

                # y = x*s (scalar engine)
                y_sb = ypool.tile([P, FREE], mybir.dt.float32)
                for cI in range(CH):
                    nc.scalar.activation(
                        y_sb[:, cI * E:(cI + 1) * E],
                        x_sb[:, cI * E:(cI + 1) * E],
                        mybir.ActivationFunctionType.Copy,
                        scale=sscl_sb[:, cI:cI + 1],
                    )

                a_sb = apool.tile([P, FREE], mybir.dt.float32)
                if pe_merge:
                    # one-hot selection rows: Sc_n[p, ch] = (topk[n*128+p]==ch)
                    sc_sb = scpool.tile([P, NRK * C], mybir.dt.float32)
                    for n in range(NRK):
                        nc.vector.tensor_scalar(
                            sc_sb[:, n * C:(n + 1) * C], iota_f[:],
                            sscl_sb[:, 2 * CH + NRK + n:2 * CH + NRK + n + 1],
                            None, op0=mybir.AluOpType.is_equal,
                        )
                    # delta[ch_chunk] = sum_n Sc_n[:, chunk]^T @ xq_n
                    for cI in range(CH):
                        ps = ppool.tile([P, E], mybir.dt.float32, space="PSUM")
                        for n in range(NRK):
                            nc.tensor.matmul(
                                ps[:],
                                sc_sb[:, n * C + cI * P:n * C + (cI + 1) * P],
                                xq_sb[:, n * E:(n + 1) * E],
                                start=(n == 0),
                                stop=(n == NRK - 1),
                            )
                        # aug = x*A + delta
                        nc.vector.scalar_tensor_tensor(
                            a_sb[:, cI * E:(cI + 1) * E],
                            x_sb[:, cI * E:(cI + 1) * E],
                            sscl_sb[:, CH + cI:CH + cI + 1],
                            ps[:],
                            op0=mybir.AluOpType.mult,
                            op1=mybir.AluOpType.add,
                        )
                else:
                    for cI in range(CH):
                        nc.vector.tensor_scalar_mul(
                            a_sb[:, cI * E:(cI + 1) * E],
                            x_sb[:, cI * E:(cI + 1) * E],
                            sscl_sb[:, CH + cI:CH + cI + 1],
                        )

                nc.sync.dma_start(
                    outs[i][0:C].rearrange("(p c) e -> p (c e)", p=P), y_sb[:]
                )
                nc.sync.dma_start(
                    outs[i][C:2 * C].rearrange("(p c) e -> p (c e)", p=P), a_sb[:]
                )
                if not pe_merge:
                    # scatter-ADD blend remainder over the aug slot's topk rows
                    for n in range(NRK):
                        nc.gpsimd.indirect_dma_start(
                            out=outs[i][:],
                            out_offset=bass.IndirectOffsetOnAxis(
                                ap=oidx_sb[:, n:n + 1], axis=0
                            ),
                            in_=xq_sb[:, n * E:(n + 1) * E],
                            in_offset=None,
                            bounds_check=2 * C - 1,
                            oob_is_err=False,
                            compute_op=mybir.AluOpType.add,
                        )

    nc.compile()
    return nc


def _get_nc(n_loc=NLOC, reps=1, bufs=3, pe_merge=True):
    key = (n_loc, reps, bufs, pe_merge)
    if key not in _CACHE:
        _CACHE[key] = _build(n_loc, reps, bufs, pe_merge)
    return _CACHE[key]


def _wrap16(stream):
    """[S] stream -> [P, S//16] int16 tile (16-wrapped, replicated per core)."""
    t = stream.reshape(S // 16, 16).T.astype(np.int16)     # [16, S//16]
    return np.tile(t, (8, 1))                              # [128, S//16]


def _prep(x, s_ca, rand_index, partner):
    """Host-side index/scale prep. Returns per-core input maps."""
    scores = np.asarray(s_ca, np.float32).reshape(N, C)
    x = np.ascontiguousarray(np.asarray(x, np.float32).reshape(N, C, E))
    rand_index = np.asarray(rand_index).astype(np.int64).reshape(N, S)
    partner = np.asarray(partner).astype(np.int64).reshape(N)

    # top-k (stable desc sort == jax.lax.top_k tie semantics)
    order = np.argsort(-scores, axis=1, kind="stable")
    topk = order[:, :S]                                    # [N, S]
    j = (np.arange(N) + 1 + partner) % N                   # partner sample

    rows = np.arange(N)
    i_loc = rows % NLOC
    s_topk = np.take_along_axis(scores, topk, axis=1)      # [N, S]

    a_v = scores.copy()
    np.put_along_axis(a_v, topk, np.float32(0.7) * s_topk, axis=1)

    sscl = np.concatenate([
        scores.reshape(N, P, CH),
        a_v.reshape(N, P, CH),
        (np.float32(0.3) * s_topk).reshape(N, NRK, P).transpose(0, 2, 1),
        topk.astype(np.float32).reshape(N, NRK, P).transpose(0, 2, 1),
    ], axis=2).astype(np.float32)                        # [N, P, 2*CH+2*NRK]

    # partner gather stream (rank order): rows in x_part flat tensor
    st_part = (i_loc[:, None] * C + rand_index).astype(np.int64)   # [N, S]
    gidx = np.empty((N, P, SW), np.int16)
    for g in range(N):
        gidx[g] = _wrap16(st_part[g])

    # scatter rows at rank slot (p, n): C + topk_idx[g, n*128+p]
    oidx = (C + topk).reshape(N, NRK, P).transpose(0, 2, 1).astype(np.int32)

    in_maps = []
    for k in range(NCORES):
        sl = slice(k * NLOC, (k + 1) * NLOC)
        in_maps.append({
            "x_own": x[sl].reshape(NLOC * C, E),
            "x_part": np.ascontiguousarray(x[j[sl]]).reshape(NLOC * C, E),
            "sscl": np.ascontiguousarray(sscl[sl]),
            "gidx": np.ascontiguousarray(gidx[sl]),
            "oidx": np.ascontiguousarray(oidx[sl]),
        })
    return in_maps


def _assemble(results):
    """Map per-core out{i} [2C, E] tensors into the full [2N, C, 16, 16]."""
    full = np.empty((2 * N, C, 16, 16), np.float32)
    for k in range(NCORES):
        for il in range(NLOC):
            oc = results[k][f"out{il}"].reshape(2, C, 16, 16)
            g = k * NLOC + il
            way, t = g // 16, g % 16
            full[way * 32 + t] = oc[0]
            full[way * 32 + 16 + t] = oc[1]
    return full


def kernel(x, s_ca, rand_index, partner, shuffle_num, _trace=False):
    from concourse import bass_utils

    assert int(shuffle_num) == S
    in_maps = _prep(x, s_ca, rand_index, partner)
    nc = _get_nc()
    res = bass_utils.run_bass_kernel_spmd(
        nc, in_maps, core_ids=list(range(NCORES)), trace=_trace
    )
    out = _assemble(res.results)
    if _trace:
        return out, res
    return out
